# revision 4
# baseline (speedup 1.0000x reference)
"""Multi-head attention Bass/Tile kernel for Trainium2, sharded over 8 NeuronCores.

v2 schedule (same math as baseline: QKV proj + 2-head row-tiled scores +
exp + ones-column ctx/rowsum matmuls + out-proj partials summed on host):

  - host packs every input into partition-major blobs so each load is ONE
    full-bus DMA (128 rows x 4-8KB contiguous); s-chunk granularity for x
    so the first projections unblock ~5us in;
  - attention starts after a small prologue; V/K/Q projections run as
    deadline-checked filler inside the attention k-loop, out-projections
    as paced filler; half-group granularity smooths PE insertion;
  - projection evictions ride the ACT engine while it would idle (qc0 +
    drain tail), DVE otherwise; eviction DMAs issue from the DVE queue so
    the SP queue stays a pure load pipe;
  - per-block softmax normalization: 2 psum copies -> 1 reciprocal ->
    1 row DMA hop -> 1 partition broadcast -> 2 muls.
"""

import numpy as np
import ml_dtypes
from collections import deque

BF16 = ml_dtypes.bfloat16

FULL = dict(S=2048, D=1024, G=8, QC=512)
N_CORES = 8
DH = 64


def build_body(nc, S, D, G, QC, repeat=1):
    import concourse.tile as tile
    from concourse import mybir
    from contextlib import ExitStack

    GF = G * DH
    KT_N = S // 128
    DT_N = D // 128
    FT_N = GF // 128
    QC_N = S // QC
    SUB = QC // 128
    JW = min(512, D)
    J_N = D // JW
    SC_W = min(512, S)
    SC_N = S // SC_W
    f32 = mybir.dt.float32
    bf16 = mybir.dt.bfloat16

    # blob layouts (see shard_inputs): x: [128, (sc, dt, SC_W)],
    # w: [128, (dt, GF)], wo: [128, (ft, D)], bq/bk: [128, FT_N], bvb: [128, GF]
    xq_d = nc.dram_tensor("xq", [128, DT_N * S], bf16, kind="ExternalInput").ap()
    xk_d = nc.dram_tensor("xk", [128, DT_N * S], bf16, kind="ExternalInput").ap()
    xv_d = nc.dram_tensor("xv", [128, DT_N * S], bf16, kind="ExternalInput").ap()
    wq_d = nc.dram_tensor("wq", [128, DT_N * GF], bf16, kind="ExternalInput").ap()
    wk_d = nc.dram_tensor("wk", [128, DT_N * GF], bf16, kind="ExternalInput").ap()
    wv_d = nc.dram_tensor("wv", [128, DT_N * GF], bf16, kind="ExternalInput").ap()
    wo_d = nc.dram_tensor("wo", [128, FT_N * D], bf16, kind="ExternalInput").ap()
    bq_d = nc.dram_tensor("bq", [128, FT_N], f32, kind="ExternalInput").ap()
    bk_d = nc.dram_tensor("bk", [128, FT_N], f32, kind="ExternalInput").ap()
    bvb_d = nc.dram_tensor("bvb", [128, GF], f32, kind="ExternalInput").ap()
    out_d = nc.dram_tensor("out", [S, D], f32, kind="ExternalOutput").ap()

    with tile.TileContext(nc) as tc, ExitStack() as ctx:
        pw = ctx.enter_context(tc.tile_pool(name="pw", bufs=3))
        pwo = ctx.enter_context(tc.tile_pool(name="pwo", bufs=1))
        pxk = ctx.enter_context(tc.tile_pool(name="pxk", bufs=SC_N))
        pxq = ctx.enter_context(tc.tile_pool(name="pxq", bufs=2))
        pxv = ctx.enter_context(tc.tile_pool(name="pxv", bufs=2))
        pqt = ctx.enter_context(tc.tile_pool(name="pqt", bufs=FT_N))
        pkt = ctx.enter_context(tc.tile_pool(name="pkt", bufs=FT_N))
        pv = ctx.enter_context(tc.tile_pool(name="pv", bufs=KT_N))
        ppt = ctx.enter_context(tc.tile_pool(name="ppt", bufs=3))
        pctx = ctx.enter_context(tc.tile_pool(name="pctx", bufs=FT_N))
        psm = ctx.enter_context(tc.tile_pool(name="psm", bufs=2))
        pout = ctx.enter_context(tc.tile_pool(name="pout", bufs=3))
        pcst = ctx.enter_context(tc.tile_pool(name="pcst", bufs=1))
        # PSUM: ST 2x2 banks + ctx 2x1 + proj/out 2x1 = 8 banks
        pst_ps = ctx.enter_context(tc.tile_pool(name="pst_ps", bufs=2, space="PSUM"))
        pctx_ps = ctx.enter_context(tc.tile_pool(name="pctx_ps", bufs=2, space="PSUM"))
        pmm_ps = ctx.enter_context(tc.tile_pool(name="pmm_ps", bufs=2, space="PSUM"))

        env = dict(
            S=S, D=D, G=G, QC=QC, GF=GF, KT_N=KT_N, DT_N=DT_N, FT_N=FT_N,
            QC_N=QC_N, SUB=SUB, JW=JW, J_N=J_N, SC_W=SC_W, SC_N=SC_N,
            xq_d=xq_d, xk_d=xk_d, xv_d=xv_d, wq_d=wq_d, wk_d=wk_d, wv_d=wv_d,
            wo_d=wo_d, bq_d=bq_d, bk_d=bk_d, bvb_d=bvb_d, out_d=out_d,
            pw=pw, pwo=pwo, pxk=pxk, pxq=pxq, pxv=pxv, pqt=pqt, pkt=pkt,
            pv=pv, ppt=ppt, pctx=pctx, psm=psm, pout=pout, pcst=pcst,
            pst_ps=pst_ps, pctx_ps=pctx_ps, pmm_ps=pmm_ps)
        for _rep in range(repeat):
            _emit_rep(nc, tc, env)
    return nc


def _emit_rep(nc, tc, env):
    from concourse import mybir
    f32 = mybir.dt.float32
    bf16 = mybir.dt.bfloat16
    EXP = mybir.ActivationFunctionType.Exp
    IDENT = mybir.ActivationFunctionType.Identity
    COPY = mybir.ActivationFunctionType.Copy
    S, D, G, QC, GF = env["S"], env["D"], env["G"], env["QC"], env["GF"]
    DT_N, FT_N, KT_N, QC_N, SUB = (
        env["DT_N"], env["FT_N"], env["KT_N"], env["QC_N"], env["SUB"])
    SC_W, SC_N, JW, J_N = env["SC_W"], env["SC_N"], env["JW"], env["J_N"]
    xq_d, xk_d, xv_d, out_d = env["xq_d"], env["xk_d"], env["xv_d"], env["out_d"]
    wq_d, wk_d, wv_d, wo_d = env["wq_d"], env["wk_d"], env["wv_d"], env["wo_d"]
    bq_d, bk_d, bvb_d = env["bq_d"], env["bk_d"], env["bvb_d"]
    pw, pwo, pxk, pxq, pxv, pqt, pkt, pv, ppt, pctx, psm, pout, pcst = (
        env["pw"], env["pwo"], env["pxk"], env["pxq"], env["pxv"], env["pqt"],
        env["pkt"], env["pv"], env["ppt"], env["pctx"], env["psm"],
        env["pout"], env["pcst"])
    pst_ps, pctx_ps, pmm_ps = env["pst_ps"], env["pctx_ps"], env["pmm_ps"]
    XCW = DT_N * SC_W          # x chunk tile width (all dt of one s-chunk)

    # ---- load DMAs: one blob DMA per (tensor, s-chunk). DMA_ENGINES
    # serializes transfers, so order = first-need order: the K/Q path for
    # the first scores, then the V path, then the rest. ----
    xv_ch = {}
    xq_ch = {}

    def get_chunk(store, pool, src, tag, sc):
        # lazy: a chunk DMA into a reused pool buffer must be EMITTED after
        # that buffer's previous readers (pool bufs=2 -> sc reuses sc-2's
        # buffer; call sites guarantee sc-2's readers were emitted).
        if sc not in store:
            t = pool.tile([128, XCW], bf16, tag=tag, name=f"{tag}{sc}")
            nc.sync.dma_start(t[:], src[:, sc * XCW:(sc + 1) * XCW])
            store[sc] = t
        return store[sc]

    wk_sb = pw.tile([128, DT_N * GF], bf16, tag="w", name="wk")
    nc.sync.dma_start(wk_sb[:], wk_d[:])
    xk_ch = [pxk.tile([128, XCW], bf16, tag="xk", name=f"xk{sc}")
             for sc in range(SC_N)]
    nc.sync.dma_start(xk_ch[0][:], xk_d[:, 0:XCW])
    wq_sb = pw.tile([128, DT_N * GF], bf16, tag="w", name="wq")
    nc.sync.dma_start(wq_sb[:], wq_d[:])
    get_chunk(xq_ch, pxq, xq_d, "xq", 0)
    bk_sb = pcst.tile([128, FT_N], f32, tag="bk")
    nc.sync.dma_start(bk_sb[:], bk_d[:])
    bq_sb = pcst.tile([128, FT_N], f32, tag="bq")
    nc.sync.dma_start(bq_sb[:], bq_d[:])
    wv_sb = pw.tile([128, DT_N * GF], bf16, tag="w", name="wv")
    nc.sync.dma_start(wv_sb[:], wv_d[:])
    get_chunk(xv_ch, pxv, xv_d, "xv", 0)
    bvb = pcst.tile([128, GF], f32, tag="bvb")
    nc.sync.dma_start(bvb[:], bvb_d[:])
    for sc in range(1, SC_N):
        nc.sync.dma_start(xk_ch[sc][:], xk_d[:, sc * XCW:(sc + 1) * XCW])
    if SC_N > 1:
        get_chunk(xv_ch, pxv, xv_d, "xv", 1)
        get_chunk(xq_ch, pxq, xq_d, "xq", 1)
    wo_sb = pwo.tile([128, FT_N * D], bf16, tag="wo")
    nc.sync.dma_start(wo_sb[:], wo_d[:])
    # warm the exp table early
    warm = pcst.tile([1, 8], f32, tag="warm")
    nc.vector.memset(warm[:], 0.0)
    nc.scalar.activation(warm[:], warm[:], EXP)

    qt_sb = [pqt.tile([128, S], bf16, tag="q", name=f"qt{i}") for i in range(FT_N)]
    kt_sb = [pkt.tile([128, S], bf16, tag="k", name=f"kt{i}") for i in range(FT_N)]
    v_sb = [pv.tile([128, G * 65], bf16, tag="v", name=f"v{i}")
            for i in range(KT_N)]
    ctx_sb = [pctx.tile([128, S], bf16, tag="ctx", name=f"ctxsb{i}")
              for i in range(FT_N)]

    # ---- projection emitters: unit -> 2 half-group steps ----
    HALF = DT_N // 2
    done = {}
    open_ps = {}
    evict_act = [True]          # qc0 + drain: evictions ride ACT

    def vproj_half(st, half):
        if half == 0:
            ps = pmm_ps.tile([128, GF], f32, tag="mm")
            open_ps[("v", st)] = ps
        else:
            ps = open_ps.pop(("v", st))
        xt = get_chunk(xv_ch, pxv, xv_d, "xv", st // (SC_W // 128))
        col = (st * 128) % SC_W
        for dt in range(half * HALF, half * HALF + HALF):
            nc.tensor.matmul(
                ps[:], xt[:, dt * SC_W + col:dt * SC_W + col + 128],
                wv_sb[:, dt * GF:(dt + 1) * GF],
                start=(dt == 0), stop=(dt == DT_N - 1))
        if half == 1:
            tv = v_sb[st][:].rearrange("p (g e) -> p g e", e=65)
            nc.vector.tensor_add(
                tv[:, :, 0:64], ps[:].rearrange("p (g d) -> p g d", d=64),
                bvb[:].rearrange("p (g d) -> p g d", d=64))
            nc.vector.memset(tv[:, :, 64:65], 1.0)

    def qkproj_half(name, ft, sc, half):
        if name == "q":
            w_sb, b_sb, dst = wq_sb, bq_sb, qt_sb
            xs = get_chunk(xq_ch, pxq, xq_d, "xq", sc)
        else:
            w_sb, b_sb, dst, xs = wk_sb, bk_sb, kt_sb, xk_ch[sc]
        key = (name, ft, sc)
        if half == 0:
            ps = pmm_ps.tile([128, SC_W], f32, tag="mm")
            open_ps[key] = ps
        else:
            ps = open_ps.pop(key)
        for dt in range(half * HALF, half * HALF + HALF):
            nc.tensor.matmul(
                ps[:], w_sb[:, dt * GF + ft * 128:dt * GF + (ft + 1) * 128],
                xs[:, dt * SC_W:(dt + 1) * SC_W],
                start=(dt == 0), stop=(dt == DT_N - 1))
        if half == 1:
            o = dst[ft][:, sc * SC_W:(sc + 1) * SC_W]
            if evict_act[0]:
                nc.scalar.activation(o, ps[:], IDENT, bias=b_sb[:, ft:ft + 1])
            else:
                nc.vector.tensor_scalar_add(o, ps[:], b_sb[:, ft:ft + 1])

    def outproj_half(qc, su, j, half):
        rows = slice(qc * QC + su * 128, qc * QC + (su + 1) * 128)
        key = ("o", qc, su, j)
        if half == 0:
            ps = pmm_ps.tile([128, JW], f32, tag="mm")
            open_ps[key] = ps
        else:
            ps = open_ps.pop(key)
        fh = FT_N // 2
        for ft in range(half * fh, half * fh + fh):
            nc.tensor.matmul(
                ps[:], ctx_sb[ft][:, rows],
                wo_sb[:, ft * D + j * JW:ft * D + (j + 1) * JW],
                start=(ft == 0), stop=(ft == FT_N - 1))
        if half == 1:
            o = pout.tile([128, JW], f32, tag="o")
            if evict_act[0]:
                nc.scalar.activation(o[:], ps[:], COPY)
            else:
                nc.vector.tensor_copy(o[:], ps[:])
            nc.sync.dma_start(out_d[rows, j * JW:(j + 1) * JW], o[:])

    # last-q-chunk out-proj, split into two self-contained psum passes so
    # the ft0/ft1 pass runs as filler inside the last two attention blocks
    # and only the ft2/ft3 pass remains for the drain. The partial stages
    # to bf16 SBUF (rounding ~0.1% of a half-sum, well inside budget).
    stage = {}

    def outproj_p1(su, j, half):
        qc = QC_N - 1
        rows = slice(qc * QC + su * 128, qc * QC + (su + 1) * 128)
        key = ("p1", su, j)
        if half == 0:
            ps = pmm_ps.tile([128, JW], f32, tag="mm")
            open_ps[key] = ps
            for ft in range(FT_N // 2):
                nc.tensor.matmul(
                    ps[:], ctx_sb[ft][:, rows],
                    wo_sb[:, ft * D + j * JW:ft * D + (j + 1) * JW],
                    start=(ft == 0), stop=(ft == FT_N // 2 - 1))
        else:
            ps = open_ps.pop(key)
            t = pout.tile([128, JW], bf16, tag="stg", bufs=SUB * J_N)
            nc.vector.tensor_copy(t[:], ps[:])
            stage[(su, j)] = t

    def outproj_p2(su, j, half):
        qc = QC_N - 1
        rows = slice(qc * QC + su * 128, qc * QC + (su + 1) * 128)
        key = ("p2", su, j)
        if half == 0:
            ps = pmm_ps.tile([128, JW], f32, tag="mm")
            open_ps[key] = ps
            for ft in range(FT_N // 2, FT_N):
                nc.tensor.matmul(
                    ps[:], ctx_sb[ft][:, rows],
                    wo_sb[:, ft * D + j * JW:ft * D + (j + 1) * JW],
                    start=(ft == FT_N // 2), stop=(ft == FT_N - 1))
        else:
            ps = open_ps.pop(key)
            o = pout.tile([128, JW], f32, tag="o")
            nc.vector.tensor_add(o[:], ps[:], stage.pop((su, j))[:])
            nc.sync.dma_start(out_d[rows, j * JW:(j + 1) * JW], o[:])

    def advance(unit):
        n = done.get(unit, 0)
        if n >= 2:
            return False
        kind = unit[0]
        if kind == "v":
            vproj_half(unit[1], n)
        elif kind in ("q", "k"):
            qkproj_half(kind, unit[1], unit[2], n)
        elif kind == "p1":
            outproj_p1(unit[1], unit[2], n)
        elif kind == "p2":
            outproj_p2(unit[1], unit[2], n)
        else:
            outproj_half(unit[1], unit[2], unit[3], n)
        done[unit] = n + 1
        return True

    def ensure(unit):
        while advance(unit):
            pass

    fill = deque()

    def pace():
        while fill:
            unit = fill[0]
            if done.get(unit, 0) >= 2:
                fill.popleft()
                continue
            advance(unit)
            if done.get(unit, 0) >= 2:
                fill.popleft()
            return True
        return False

    for st in range(4, KT_N):
        fill.append(("v", st))
    for ft in range(1, FT_N):
        for sc in range(SC_N):
            fill.append(("k", ft, sc))
        fill.append(("q", ft, 0))
    for sc in range(1, SC_N):
        for ft in range(FT_N):
            fill.append(("q", ft, sc))

    # ---- prologue: just enough for the first attention steps ----
    ensure(("k", 0, 0))
    ensure(("q", 0, 0))
    for st in range(4):
        ensure(("v", st))

    def ctx_mm(ctx_h, ft, kt, pt):
        for h in range(2):
            h65 = (2 * ft + h) * 65
            nc.tensor.matmul(
                ctx_h[h][:, :], v_sb[kt][:, h65:h65 + 65],
                pt[:, h * QC:(h + 1) * QC],
                start=(kt == 0), stop=(kt == KT_N - 1),
                skip_group_check=True)

    # ---- attention, q-chunk major with deadline + quota-paced filler:
    # each q-chunk spreads the currently-available filler halves evenly
    # over its 64 k-steps so PE neither starves late nor delays ACT ----
    step = 0
    credit = 0.0
    per_step = 0.0
    for qc in range(QC_N):
        qs = slice(qc * QC, (qc + 1) * QC)
        if 1 <= qc and qc + 1 < SC_N:
            # prefetch xq chunk qc+1 (its buffer's readers, Q(*, qc-1),
            # were all ensured during the previous q-chunk's blocks)
            get_chunk(xq_ch, pxq, xq_d, "xq", qc + 1)
        if qc > 0:
            halves = sum(2 - done.get(u, 0) for u in fill)
            per_step = halves / (KT_N * FT_N)
            credit = 0.0
        for ft in range(FT_N):
            ensure(("q", ft, qc))
            ctx_h = [pctx_ps.tile([65, QC], f32, tag="ctx", name=f"ctxps{h}")
                     for h in range(2)]
            pt_tiles = []
            for kt in range(KT_N):
                if qc > 0:
                    credit += per_step
                    while credit >= 1.0:
                        credit -= 1.0
                        if not pace():
                            credit = 0.0
                # prefetch upcoming xv chunks once their buffer's previous
                # readers (V s-tiles of chunk sc-2) are all emitted
                if qc == 0 and ft == 0 and kt in (6, 10):
                    get_chunk(xv_ch, pxv, xv_d, "xv", kt // 4 + 1)
                ensure(("k", ft, (kt * 128) // SC_W))
                st_ps = pst_ps.tile([128, 2 * QC], f32, tag="st")
                ks = slice(kt * 128, (kt + 1) * 128)
                for h in range(2):
                    hp = slice(h * 64, (h + 1) * 64)
                    nc.tensor.matmul(
                        st_ps[:, h * QC:(h + 1) * QC],
                        kt_sb[ft][hp, ks], qt_sb[ft][hp, qs],
                        start=True, stop=True, tile_position=(h * 64, 0))
                pt = ppt.tile([128, 2 * QC], bf16, tag="pt")
                nc.scalar.activation(pt[:], st_ps[:], EXP, scale=0.125)
                pt_tiles.append(pt)
                if kt >= 1:
                    ensure(("v", kt - 1))
                    ctx_mm(ctx_h, ft, kt - 1, pt_tiles[kt - 1])
                step += 1
            ensure(("v", KT_N - 1))
            ctx_mm(ctx_h, ft, KT_N - 1, pt_tiles[KT_N - 1])
            # ---- eviction: 2 copies -> 1 recip -> 1 DVE-DMA hop -> 1
            # broadcast -> 2 muls (+1 DVE-DMA partition shift) ----
            ctxr = psm.tile([65, 2 * QC], f32, tag="ctxr", bufs=2)
            for h in range(2):
                nc.vector.tensor_copy(ctxr[:, h * QC:(h + 1) * QC], ctx_h[h][:, :])
            # recip -> partition-0 hop -> broadcast, all inside the bc tile
            bc = psm.tile([128, 2 * QC], f32, tag="bc", bufs=1)
            nc.vector.reciprocal(bc[64:65, :], ctxr[64:65, :])
            nc.gpsimd.dma_start(bc[0:1, :], bc[64:65, :])
            nc.gpsimd.partition_broadcast(bc[:, :], bc[0:1, :])
            nc.vector.tensor_mul(
                ctx_sb[ft][0:64, qs], ctxr[0:64, 0:QC], bc[0:64, 0:QC])
            oddt = psm.tile([64, QC], bf16, tag="oddt")
            nc.vector.tensor_mul(
                oddt[:, :], ctxr[0:64, QC:2 * QC], bc[0:64, QC:2 * QC])
            nc.gpsimd.dma_start(ctx_sb[ft][64:128, qs], oddt[:, :])
            if qc == QC_N - 1 and ft == 1:
                # ctx for ft0/ft1 final: first out-proj pass becomes filler
                # for the remaining two blocks
                for su in range(SUB):
                    for j in range(J_N):
                        fill.append(("p1", su, j))
                halves = sum(2 - done.get(u, 0) for u in fill)
                per_step = halves / (2 * KT_N)
                credit = 0.0
        if qc == 0:
            evict_act[0] = False
        if qc == QC_N - 1:
            for su in range(SUB):
                for j in range(J_N):
                    fill.append(("p2", su, j))
        else:
            for su in range(SUB):
                for j in range(J_N):
                    fill.append(("o", qc, su, j))
    evict_act[0] = True
    while fill:
        pace()
    return nc


def build_nc(S=None, D=None, G=None, QC=None, num_devices=N_CORES, repeat=1):
    cfg = dict(FULL)
    for k, v in (("S", S), ("D", D), ("G", G), ("QC", QC)):
        if v is not None:
            cfg[k] = v
    from concourse import bacc
    nc = bacc.Bacc("TRN2", target_bir_lowering=False, debug=False,
                   num_devices=num_devices)
    build_body(nc, **cfg, repeat=repeat)
    nc.compile()
    return nc


def _blob_x(xT, DT_N, SC_N, SC_W):
    # [D, S] -> [128, (sc, dt, SC_W)]
    Dd, Ss = xT.shape
    return np.ascontiguousarray(
        xT.reshape(DT_N, 128, SC_N, SC_W).transpose(1, 2, 0, 3).reshape(
            128, DT_N * Ss))


def _blob_w(wT, DT_N, GF):
    # [D, GF] -> [128, (dt, GF)]
    return np.ascontiguousarray(
        wT.reshape(DT_N, 128, GF).transpose(1, 0, 2).reshape(128, DT_N * GF))


def shard_inputs(q, k, v, Wq, bq, Wk, bk, Wv, bv, Wo, bo,
                 S=None, D=None, G=None, n_cores=N_CORES):
    S = S or FULL["S"]
    D = D or FULL["D"]
    G = G or FULL["G"]
    GF = G * DH
    DT_N = D // 128
    FT_N = GF // 128
    SC_W = min(512, S)
    SC_N = S // SC_W
    n_groups = Wq.shape[0] // GF
    in_maps = []
    for c in range(n_cores):
        b, g = divmod(c, n_groups)
        gs = slice(g * GF, (g + 1) * GF)
        m = {
            "xq": _blob_x(np.asarray(q[b]).T.astype(BF16), DT_N, SC_N, SC_W),
            "xk": _blob_x(np.asarray(k[b]).T.astype(BF16), DT_N, SC_N, SC_W),
            "xv": _blob_x(np.asarray(v[b]).T.astype(BF16), DT_N, SC_N, SC_W),
            "wq": _blob_w(np.asarray(Wq)[gs, :].T.astype(BF16), DT_N, GF),
            "wk": _blob_w(np.asarray(Wk)[gs, :].T.astype(BF16), DT_N, GF),
            "wv": _blob_w(np.asarray(Wv)[gs, :].T.astype(BF16), DT_N, GF),
            "wo": _blob_w(np.asarray(Wo)[:, gs].T.astype(BF16), FT_N, D),
            "bq": np.ascontiguousarray(
                np.asarray(bq)[gs].reshape(FT_N, 128).T.astype(np.float32)),
            "bk": np.ascontiguousarray(
                np.asarray(bk)[gs].reshape(FT_N, 128).T.astype(np.float32)),
            "bvb": np.ascontiguousarray(
                np.tile(np.asarray(bv)[gs].astype(np.float32), (128, 1))),
        }
        in_maps.append(m)
    return in_maps


def gather_outputs(results, bo, n_groups=2):
    n_b = len(results) // n_groups
    outs = []
    for b in range(n_b):
        acc = results[b * n_groups]["out"].astype(np.float32)
        for g in range(1, n_groups):
            acc = acc + results[b * n_groups + g]["out"]
        outs.append(acc + np.asarray(bo, np.float32)[None, :])
    return np.stack(outs, axis=0)


_NC_CACHE = {}


def kernel(q, k, v, Wq, bq, Wk, bk, Wv, bv, Wo, bo):
    from concourse.bass_utils import run_bass_kernel_spmd
    key = "full"
    if key not in _NC_CACHE:
        _NC_CACHE[key] = build_nc()
    nc = _NC_CACHE[key]
    in_maps = shard_inputs(q, k, v, Wq, bq, Wk, bk, Wv, bv, Wo, bo)
    res = run_bass_kernel_spmd(nc, in_maps, core_ids=list(range(N_CORES)))
    return gather_outputs(res.results, bo)


# revision 6
# speedup vs baseline: 1.0079x; 1.0079x over previous
"""Multi-head attention Bass/Tile kernel for Trainium2, sharded over 8 NeuronCores.

v2 schedule (same math as baseline: QKV proj + 2-head row-tiled scores +
exp + ones-column ctx/rowsum matmuls + out-proj partials summed on host):

  - host packs every input into partition-major blobs so each load is ONE
    full-bus DMA (128 rows x 4-8KB contiguous); s-chunk granularity for x
    so the first projections unblock ~5us in;
  - attention starts after a small prologue; V/K/Q projections run as
    deadline-checked filler inside the attention k-loop, out-projections
    as paced filler; half-group granularity smooths PE insertion;
  - projection evictions ride the ACT engine while it would idle (qc0 +
    drain tail), DVE otherwise; eviction DMAs issue from the DVE queue so
    the SP queue stays a pure load pipe;
  - per-block softmax normalization: 2 psum copies -> 1 reciprocal ->
    1 row DMA hop -> 1 partition broadcast -> 2 muls.
"""

import numpy as np
import ml_dtypes
from collections import deque

BF16 = ml_dtypes.bfloat16

FULL = dict(S=2048, D=1024, G=8, QC=512)
N_CORES = 8
DH = 64


def build_body(nc, S, D, G, QC, repeat=1):
    import concourse.tile as tile
    from concourse import mybir
    from contextlib import ExitStack

    GF = G * DH
    KT_N = S // 128
    DT_N = D // 128
    FT_N = GF // 128
    QC_N = S // QC
    SUB = QC // 128
    JW = min(512, D)
    J_N = D // JW
    SC_W = min(512, S)
    SC_N = S // SC_W
    f32 = mybir.dt.float32
    bf16 = mybir.dt.bfloat16

    # blob layouts (see shard_inputs): x: [128, (sc, dt, SC_W)],
    # w: [128, (dt, GF)], wo: [128, (ft, D)], bq/bk: [128, FT_N], bvb: [128, GF]
    xq_d = nc.dram_tensor("xq", [128, DT_N * S], bf16, kind="ExternalInput").ap()
    xk_d = nc.dram_tensor("xk", [128, DT_N * S], bf16, kind="ExternalInput").ap()
    xv_d = nc.dram_tensor("xv", [128, DT_N * S], bf16, kind="ExternalInput").ap()
    wq_d = nc.dram_tensor("wq", [128, DT_N * GF], bf16, kind="ExternalInput").ap()
    wk_d = nc.dram_tensor("wk", [128, DT_N * GF], bf16, kind="ExternalInput").ap()
    wv_d = nc.dram_tensor("wv", [128, DT_N * GF], bf16, kind="ExternalInput").ap()
    wo_d = nc.dram_tensor("wo", [128, FT_N * D], bf16, kind="ExternalInput").ap()
    bq_d = nc.dram_tensor("bq", [128, FT_N], f32, kind="ExternalInput").ap()
    bk_d = nc.dram_tensor("bk", [128, FT_N], f32, kind="ExternalInput").ap()
    bvb_d = nc.dram_tensor("bvb", [128, GF], f32, kind="ExternalInput").ap()
    out_d = nc.dram_tensor("out", [S, D], f32, kind="ExternalOutput").ap()

    with tile.TileContext(nc) as tc, ExitStack() as ctx:
        pw = ctx.enter_context(tc.tile_pool(name="pw", bufs=3))
        pwo = ctx.enter_context(tc.tile_pool(name="pwo", bufs=1))
        pxk = ctx.enter_context(tc.tile_pool(name="pxk", bufs=SC_N))
        pxq = ctx.enter_context(tc.tile_pool(name="pxq", bufs=2))
        pxv = ctx.enter_context(tc.tile_pool(name="pxv", bufs=2))
        pqt = ctx.enter_context(tc.tile_pool(name="pqt", bufs=FT_N))
        pkt = ctx.enter_context(tc.tile_pool(name="pkt", bufs=FT_N))
        pv = ctx.enter_context(tc.tile_pool(name="pv", bufs=KT_N))
        ppt = ctx.enter_context(tc.tile_pool(name="ppt", bufs=4))
        pctx = ctx.enter_context(tc.tile_pool(name="pctx", bufs=FT_N))
        psm = ctx.enter_context(tc.tile_pool(name="psm", bufs=2))
        pout = ctx.enter_context(tc.tile_pool(name="pout", bufs=3))
        pcst = ctx.enter_context(tc.tile_pool(name="pcst", bufs=1))
        # PSUM: ST 2x2 banks + ctx 2x1 + proj/out 2x1 = 8 banks
        pst_ps = ctx.enter_context(tc.tile_pool(name="pst_ps", bufs=2, space="PSUM"))
        pctx_ps = ctx.enter_context(tc.tile_pool(name="pctx_ps", bufs=2, space="PSUM"))
        pmm_ps = ctx.enter_context(tc.tile_pool(name="pmm_ps", bufs=2, space="PSUM"))

        env = dict(
            S=S, D=D, G=G, QC=QC, GF=GF, KT_N=KT_N, DT_N=DT_N, FT_N=FT_N,
            QC_N=QC_N, SUB=SUB, JW=JW, J_N=J_N, SC_W=SC_W, SC_N=SC_N,
            xq_d=xq_d, xk_d=xk_d, xv_d=xv_d, wq_d=wq_d, wk_d=wk_d, wv_d=wv_d,
            wo_d=wo_d, bq_d=bq_d, bk_d=bk_d, bvb_d=bvb_d, out_d=out_d,
            pw=pw, pwo=pwo, pxk=pxk, pxq=pxq, pxv=pxv, pqt=pqt, pkt=pkt,
            pv=pv, ppt=ppt, pctx=pctx, psm=psm, pout=pout, pcst=pcst,
            pst_ps=pst_ps, pctx_ps=pctx_ps, pmm_ps=pmm_ps)
        for _rep in range(repeat):
            _emit_rep(nc, tc, env)
    return nc


def _emit_rep(nc, tc, env):
    from concourse import mybir
    f32 = mybir.dt.float32
    bf16 = mybir.dt.bfloat16
    EXP = mybir.ActivationFunctionType.Exp
    IDENT = mybir.ActivationFunctionType.Identity
    COPY = mybir.ActivationFunctionType.Copy
    S, D, G, QC, GF = env["S"], env["D"], env["G"], env["QC"], env["GF"]
    DT_N, FT_N, KT_N, QC_N, SUB = (
        env["DT_N"], env["FT_N"], env["KT_N"], env["QC_N"], env["SUB"])
    SC_W, SC_N, JW, J_N = env["SC_W"], env["SC_N"], env["JW"], env["J_N"]
    xq_d, xk_d, xv_d, out_d = env["xq_d"], env["xk_d"], env["xv_d"], env["out_d"]
    wq_d, wk_d, wv_d, wo_d = env["wq_d"], env["wk_d"], env["wv_d"], env["wo_d"]
    bq_d, bk_d, bvb_d = env["bq_d"], env["bk_d"], env["bvb_d"]
    pw, pwo, pxk, pxq, pxv, pqt, pkt, pv, ppt, pctx, psm, pout, pcst = (
        env["pw"], env["pwo"], env["pxk"], env["pxq"], env["pxv"], env["pqt"],
        env["pkt"], env["pv"], env["ppt"], env["pctx"], env["psm"],
        env["pout"], env["pcst"])
    pst_ps, pctx_ps, pmm_ps = env["pst_ps"], env["pctx_ps"], env["pmm_ps"]
    XCW = DT_N * SC_W          # x chunk tile width (all dt of one s-chunk)

    # ---- load DMAs: one blob DMA per (tensor, s-chunk). DMA_ENGINES
    # serializes transfers, so order = first-need order: the K/Q path for
    # the first scores, then the V path, then the rest. ----
    xv_ch = {}
    xq_ch = {}

    def get_chunk(store, pool, src, tag, sc):
        # lazy: a chunk DMA into a reused pool buffer must be EMITTED after
        # that buffer's previous readers (pool bufs=2 -> sc reuses sc-2's
        # buffer; call sites guarantee sc-2's readers were emitted).
        if sc not in store:
            t = pool.tile([128, XCW], bf16, tag=tag, name=f"{tag}{sc}")
            nc.sync.dma_start(t[:], src[:, sc * XCW:(sc + 1) * XCW])
            store[sc] = t
        return store[sc]

    wk_sb = pw.tile([128, DT_N * GF], bf16, tag="w", name="wk")
    nc.sync.dma_start(wk_sb[:], wk_d[:])
    xk_ch = [pxk.tile([128, XCW], bf16, tag="xk", name=f"xk{sc}")
             for sc in range(SC_N)]
    nc.sync.dma_start(xk_ch[0][:], xk_d[:, 0:XCW])
    wq_sb = pw.tile([128, DT_N * GF], bf16, tag="w", name="wq")
    nc.sync.dma_start(wq_sb[:], wq_d[:])
    get_chunk(xq_ch, pxq, xq_d, "xq", 0)
    bk_sb = pcst.tile([128, FT_N], f32, tag="bk")
    nc.sync.dma_start(bk_sb[:], bk_d[:])
    bq_sb = pcst.tile([128, FT_N], f32, tag="bq")
    nc.sync.dma_start(bq_sb[:], bq_d[:])
    wv_sb = pw.tile([128, DT_N * GF], bf16, tag="w", name="wv")
    nc.sync.dma_start(wv_sb[:], wv_d[:])
    get_chunk(xv_ch, pxv, xv_d, "xv", 0)
    bvb = pcst.tile([128, GF], f32, tag="bvb")
    nc.sync.dma_start(bvb[:], bvb_d[:])
    for sc in range(1, SC_N):
        nc.sync.dma_start(xk_ch[sc][:], xk_d[:, sc * XCW:(sc + 1) * XCW])
    if SC_N > 1:
        get_chunk(xv_ch, pxv, xv_d, "xv", 1)
        get_chunk(xq_ch, pxq, xq_d, "xq", 1)
    wo_sb = pwo.tile([128, FT_N * D], bf16, tag="wo")
    nc.sync.dma_start(wo_sb[:], wo_d[:])
    # warm the exp table early
    warm = pcst.tile([1, 8], f32, tag="warm")
    nc.vector.memset(warm[:], 0.0)
    nc.scalar.activation(warm[:], warm[:], EXP)

    qt_sb = [pqt.tile([128, S], bf16, tag="q", name=f"qt{i}") for i in range(FT_N)]
    kt_sb = [pkt.tile([128, S], bf16, tag="k", name=f"kt{i}") for i in range(FT_N)]
    v_sb = [pv.tile([128, G * 65], bf16, tag="v", name=f"v{i}")
            for i in range(KT_N)]
    ctx_sb = [pctx.tile([128, S], bf16, tag="ctx", name=f"ctxsb{i}")
              for i in range(FT_N)]

    # ---- projection emitters: unit -> 2 half-group steps ----
    HALF = DT_N // 2
    done = {}
    open_ps = {}
    evict_act = [True]          # qc0 + drain: evictions ride ACT

    def vproj_half(st, half):
        if half == 0:
            ps = pmm_ps.tile([128, GF], f32, tag="mm")
            open_ps[("v", st)] = ps
        else:
            ps = open_ps.pop(("v", st))
        xt = get_chunk(xv_ch, pxv, xv_d, "xv", st // (SC_W // 128))
        col = (st * 128) % SC_W
        for dt in range(half * HALF, half * HALF + HALF):
            nc.tensor.matmul(
                ps[:], xt[:, dt * SC_W + col:dt * SC_W + col + 128],
                wv_sb[:, dt * GF:(dt + 1) * GF],
                start=(dt == 0), stop=(dt == DT_N - 1))
        if half == 1:
            tv = v_sb[st][:].rearrange("p (g e) -> p g e", e=65)
            nc.vector.tensor_add(
                tv[:, :, 0:64], ps[:].rearrange("p (g d) -> p g d", d=64),
                bvb[:].rearrange("p (g d) -> p g d", d=64))
            nc.vector.memset(tv[:, :, 64:65], 1.0)

    def qkproj_half(name, ft, sc, half):
        if name == "q":
            w_sb, b_sb, dst = wq_sb, bq_sb, qt_sb
            xs = get_chunk(xq_ch, pxq, xq_d, "xq", sc)
        else:
            w_sb, b_sb, dst, xs = wk_sb, bk_sb, kt_sb, xk_ch[sc]
        key = (name, ft, sc)
        if half == 0:
            ps = pmm_ps.tile([128, SC_W], f32, tag="mm")
            open_ps[key] = ps
        else:
            ps = open_ps.pop(key)
        for dt in range(half * HALF, half * HALF + HALF):
            nc.tensor.matmul(
                ps[:], w_sb[:, dt * GF + ft * 128:dt * GF + (ft + 1) * 128],
                xs[:, dt * SC_W:(dt + 1) * SC_W],
                start=(dt == 0), stop=(dt == DT_N - 1))
        if half == 1:
            o = dst[ft][:, sc * SC_W:(sc + 1) * SC_W]
            if evict_act[0]:
                nc.scalar.activation(o, ps[:], IDENT, bias=b_sb[:, ft:ft + 1])
            else:
                nc.vector.tensor_scalar_add(o, ps[:], b_sb[:, ft:ft + 1])

    def outproj_half(qc, su, j, half):
        rows = slice(qc * QC + su * 128, qc * QC + (su + 1) * 128)
        key = ("o", qc, su, j)
        if half == 0:
            ps = pmm_ps.tile([128, JW], f32, tag="mm")
            open_ps[key] = ps
        else:
            ps = open_ps.pop(key)
        fh = FT_N // 2
        for ft in range(half * fh, half * fh + fh):
            nc.tensor.matmul(
                ps[:], ctx_sb[ft][:, rows],
                wo_sb[:, ft * D + j * JW:ft * D + (j + 1) * JW],
                start=(ft == 0), stop=(ft == FT_N - 1))
        if half == 1:
            o = pout.tile([128, JW], f32, tag="o")
            if evict_act[0]:
                nc.scalar.activation(o[:], ps[:], COPY)
            else:
                nc.vector.tensor_copy(o[:], ps[:])
            nc.sync.dma_start(out_d[rows, j * JW:(j + 1) * JW], o[:])

    # last-q-chunk out-proj, split into two self-contained psum passes so
    # the ft0/ft1 pass runs as filler inside the last two attention blocks
    # and only the ft2/ft3 pass remains for the drain. The partial stages
    # to bf16 SBUF (rounding ~0.1% of a half-sum, well inside budget).
    stage = {}

    def outproj_p1(su, j, half):
        qc = QC_N - 1
        rows = slice(qc * QC + su * 128, qc * QC + (su + 1) * 128)
        key = ("p1", su, j)
        if half == 0:
            ps = pmm_ps.tile([128, JW], f32, tag="mm")
            open_ps[key] = ps
            for ft in range(FT_N // 2):
                nc.tensor.matmul(
                    ps[:], ctx_sb[ft][:, rows],
                    wo_sb[:, ft * D + j * JW:ft * D + (j + 1) * JW],
                    start=(ft == 0), stop=(ft == FT_N // 2 - 1))
        else:
            ps = open_ps.pop(key)
            t = pout.tile([128, JW], bf16, tag="stg", bufs=SUB * J_N)
            nc.vector.tensor_copy(t[:], ps[:])
            stage[(su, j)] = t

    def outproj_p2(su, j, half):
        qc = QC_N - 1
        rows = slice(qc * QC + su * 128, qc * QC + (su + 1) * 128)
        key = ("p2", su, j)
        if half == 0:
            ps = pmm_ps.tile([128, JW], f32, tag="mm")
            open_ps[key] = ps
            for ft in range(FT_N // 2, FT_N):
                nc.tensor.matmul(
                    ps[:], ctx_sb[ft][:, rows],
                    wo_sb[:, ft * D + j * JW:ft * D + (j + 1) * JW],
                    start=(ft == FT_N // 2), stop=(ft == FT_N - 1))
        else:
            ps = open_ps.pop(key)
            o = pout.tile([128, JW], f32, tag="o")
            nc.vector.tensor_add(o[:], ps[:], stage.pop((su, j))[:])
            nc.sync.dma_start(out_d[rows, j * JW:(j + 1) * JW], o[:])

    def advance(unit):
        n = done.get(unit, 0)
        if n >= 2:
            return False
        kind = unit[0]
        if kind == "v":
            vproj_half(unit[1], n)
        elif kind in ("q", "k"):
            qkproj_half(kind, unit[1], unit[2], n)
        elif kind == "p1":
            outproj_p1(unit[1], unit[2], n)
        elif kind == "p2":
            outproj_p2(unit[1], unit[2], n)
        else:
            outproj_half(unit[1], unit[2], unit[3], n)
        done[unit] = n + 1
        return True

    def ensure(unit):
        while advance(unit):
            pass

    fill = deque()

    def pace():
        while fill:
            unit = fill[0]
            if done.get(unit, 0) >= 2:
                fill.popleft()
                continue
            advance(unit)
            if done.get(unit, 0) >= 2:
                fill.popleft()
            return True
        return False

    for st in range(4, KT_N):
        fill.append(("v", st))
    for ft in range(1, FT_N):
        for sc in range(SC_N):
            fill.append(("k", ft, sc))
        fill.append(("q", ft, 0))
    for sc in range(1, SC_N):
        for ft in range(FT_N):
            fill.append(("q", ft, sc))

    # ---- prologue: just enough for the first attention steps ----
    ensure(("k", 0, 0))
    ensure(("q", 0, 0))
    for st in range(4):
        ensure(("v", st))

    def ctx_mm(ctx_h, ft, kt, pt):
        for h in range(2):
            h65 = (2 * ft + h) * 65
            nc.tensor.matmul(
                ctx_h[h][:, :], v_sb[kt][:, h65:h65 + 65],
                pt[:, h * QC:(h + 1) * QC],
                start=(kt == 0), stop=(kt == KT_N - 1),
                skip_group_check=True)

    # ---- attention, q-chunk major with deadline + quota-paced filler:
    # each q-chunk spreads the currently-available filler halves evenly
    # over its 64 k-steps so PE neither starves late nor delays ACT ----
    step = 0
    credit = 0.0
    per_step = 0.0
    for qc in range(QC_N):
        qs = slice(qc * QC, (qc + 1) * QC)
        if 1 <= qc and qc + 1 < SC_N:
            # prefetch xq chunk qc+1 (its buffer's readers, Q(*, qc-1),
            # were all ensured during the previous q-chunk's blocks)
            get_chunk(xq_ch, pxq, xq_d, "xq", qc + 1)
        if qc > 0:
            halves = sum(2 - done.get(u, 0) for u in fill)
            per_step = halves / (KT_N * FT_N)
            credit = 0.0
        for ft in range(FT_N):
            ensure(("q", ft, qc))
            ctx_h = [pctx_ps.tile([65, QC], f32, tag="ctx", name=f"ctxps{h}")
                     for h in range(2)]
            pt_tiles = []
            for kt in range(KT_N):
                # prefetch upcoming xv chunks once their buffer's previous
                # readers (V s-tiles of chunk sc-2) are all emitted
                if qc == 0 and ft == 0 and kt in (6, 10):
                    get_chunk(xv_ch, pxv, xv_d, "xv", kt // 4 + 1)
                ensure(("k", ft, (kt * 128) // SC_W))
                st_ps = pst_ps.tile([128, 2 * QC], f32, tag="st")
                ks = slice(kt * 128, (kt + 1) * 128)
                for h in range(2):
                    hp = slice(h * 64, (h + 1) * 64)
                    nc.tensor.matmul(
                        st_ps[:, h * QC:(h + 1) * QC],
                        kt_sb[ft][hp, ks], qt_sb[ft][hp, qs],
                        start=True, stop=True, tile_position=(h * 64, 0))
                pt = ppt.tile([128, 2 * QC], bf16, tag="pt")
                nc.scalar.activation(pt[:], st_ps[:], EXP, scale=0.125)
                pt_tiles.append(pt)
                # paced filler sits BEHIND the scores pair so exp's input is
                # never delayed by filler bursts
                if qc > 0:
                    credit += per_step
                    while credit >= 1.0:
                        credit -= 1.0
                        if not pace():
                            credit = 0.0
                if kt >= 2:
                    ensure(("v", kt - 2))
                    ctx_mm(ctx_h, ft, kt - 2, pt_tiles[kt - 2])
                step += 1
            for kt in (KT_N - 2, KT_N - 1):
                ensure(("v", kt))
                ctx_mm(ctx_h, ft, kt, pt_tiles[kt])
            # ---- eviction: 2 copies -> 1 recip -> 1 DVE-DMA hop -> 1
            # broadcast -> 2 muls (+1 DVE-DMA partition shift) ----
            ctxr = psm.tile([65, 2 * QC], f32, tag="ctxr", bufs=2)
            for h in range(2):
                nc.vector.tensor_copy(ctxr[:, h * QC:(h + 1) * QC], ctx_h[h][:, :])
            # recip -> partition-0 hop -> broadcast, all inside the bc tile
            bc = psm.tile([128, 2 * QC], f32, tag="bc", bufs=1)
            nc.vector.reciprocal(bc[64:65, :], ctxr[64:65, :])
            nc.gpsimd.dma_start(bc[0:1, :], bc[64:65, :])
            nc.gpsimd.partition_broadcast(bc[:, :], bc[0:1, :])
            nc.vector.tensor_mul(
                ctx_sb[ft][0:64, qs], ctxr[0:64, 0:QC], bc[0:64, 0:QC])
            oddt = psm.tile([64, QC], bf16, tag="oddt")
            nc.vector.tensor_mul(
                oddt[:, :], ctxr[0:64, QC:2 * QC], bc[0:64, QC:2 * QC])
            nc.gpsimd.dma_start(ctx_sb[ft][64:128, qs], oddt[:, :])
            if qc == QC_N - 1 and ft == 1:
                # ctx for ft0/ft1 final: first out-proj pass becomes filler
                # for the remaining two blocks
                for su in range(SUB):
                    for j in range(J_N):
                        fill.append(("p1", su, j))
                halves = sum(2 - done.get(u, 0) for u in fill)
                per_step = halves / (2 * KT_N)
                credit = 0.0
        if qc == 0:
            evict_act[0] = False
        if qc == QC_N - 1:
            for su in range(SUB):
                for j in range(J_N):
                    fill.append(("p2", su, j))
        else:
            for su in range(SUB):
                for j in range(J_N):
                    fill.append(("o", qc, su, j))
    evict_act[0] = True
    while fill:
        pace()
    return nc


def build_nc(S=None, D=None, G=None, QC=None, num_devices=N_CORES, repeat=1):
    cfg = dict(FULL)
    for k, v in (("S", S), ("D", D), ("G", G), ("QC", QC)):
        if v is not None:
            cfg[k] = v
    from concourse import bacc
    nc = bacc.Bacc("TRN2", target_bir_lowering=False, debug=False,
                   num_devices=num_devices)
    build_body(nc, **cfg, repeat=repeat)
    nc.compile()
    return nc


def _blob_x(xT, DT_N, SC_N, SC_W):
    # [D, S] -> [128, (sc, dt, SC_W)]
    Dd, Ss = xT.shape
    return np.ascontiguousarray(
        xT.reshape(DT_N, 128, SC_N, SC_W).transpose(1, 2, 0, 3).reshape(
            128, DT_N * Ss))


def _blob_w(wT, DT_N, GF):
    # [D, GF] -> [128, (dt, GF)]
    return np.ascontiguousarray(
        wT.reshape(DT_N, 128, GF).transpose(1, 0, 2).reshape(128, DT_N * GF))


def shard_inputs(q, k, v, Wq, bq, Wk, bk, Wv, bv, Wo, bo,
                 S=None, D=None, G=None, n_cores=N_CORES):
    S = S or FULL["S"]
    D = D or FULL["D"]
    G = G or FULL["G"]
    GF = G * DH
    DT_N = D // 128
    FT_N = GF // 128
    SC_W = min(512, S)
    SC_N = S // SC_W
    n_groups = Wq.shape[0] // GF
    in_maps = []
    for c in range(n_cores):
        b, g = divmod(c, n_groups)
        gs = slice(g * GF, (g + 1) * GF)
        m = {
            "xq": _blob_x(np.asarray(q[b]).T.astype(BF16), DT_N, SC_N, SC_W),
            "xk": _blob_x(np.asarray(k[b]).T.astype(BF16), DT_N, SC_N, SC_W),
            "xv": _blob_x(np.asarray(v[b]).T.astype(BF16), DT_N, SC_N, SC_W),
            "wq": _blob_w(np.asarray(Wq)[gs, :].T.astype(BF16), DT_N, GF),
            "wk": _blob_w(np.asarray(Wk)[gs, :].T.astype(BF16), DT_N, GF),
            "wv": _blob_w(np.asarray(Wv)[gs, :].T.astype(BF16), DT_N, GF),
            "wo": _blob_w(np.asarray(Wo)[:, gs].T.astype(BF16), FT_N, D),
            "bq": np.ascontiguousarray(
                np.asarray(bq)[gs].reshape(FT_N, 128).T.astype(np.float32)),
            "bk": np.ascontiguousarray(
                np.asarray(bk)[gs].reshape(FT_N, 128).T.astype(np.float32)),
            "bvb": np.ascontiguousarray(
                np.tile(np.asarray(bv)[gs].astype(np.float32), (128, 1))),
        }
        in_maps.append(m)
    return in_maps


def gather_outputs(results, bo, n_groups=2):
    n_b = len(results) // n_groups
    outs = []
    for b in range(n_b):
        acc = results[b * n_groups]["out"].astype(np.float32)
        for g in range(1, n_groups):
            acc = acc + results[b * n_groups + g]["out"]
        outs.append(acc + np.asarray(bo, np.float32)[None, :])
    return np.stack(outs, axis=0)


_NC_CACHE = {}


def kernel(q, k, v, Wq, bq, Wk, bk, Wv, bv, Wo, bo):
    from concourse.bass_utils import run_bass_kernel_spmd
    key = "full"
    if key not in _NC_CACHE:
        _NC_CACHE[key] = build_nc()
    nc = _NC_CACHE[key]
    in_maps = shard_inputs(q, k, v, Wq, bq, Wk, bk, Wv, bv, Wo, bo)
    res = run_bass_kernel_spmd(nc, in_maps, core_ids=list(range(N_CORES)))
    return gather_outputs(res.results, bo)
